# revision 33
# baseline (speedup 1.0000x reference)
"""Trainium2 Bass kernel for nn_AttentionLayer (B=8, H=W=64, C=256, D=128).

Strategy: data-parallel over batch B=8 across the 8 NeuronCores (attention is
independent per batch element). Per core, for its batch element's x [L=4096,
C=256]:

  phase 1, two passes: pass A PE-transposes x tiles (f32) -> resident xT
      and projects k^T [D, 512] (f32r) for ALL chunks first (kT is what
      gates chunk-0 attention); pass B projects q^T and v^T from the
      resident xT (v^T -> bf16 -> PE-transposed v tiles). PSUM->SBUF xT/vT
      copies ride the ACT engine (idle during the prologue); q/k copies
      ride the DVE.
  phase 2 (per 512-wide Lq chunk, per PAIR of 128-row Lk tiles, one global
      software pipeline across all chunks):
        MM1: S^T pair   = k_tile @ q_chunk^T x2   (PE, f32r, into a 2-bank
                          PSUM pair tile)
        exp: P~^T pair  = exp(S^T pair)           (ONE ACT op per pair ->
                          bf16; halves the ACT instruction count, which is
                          the bottleneck engine)
        MM2: A~^T      += v_tile^T @ P~^T tile    (PE, bf16 operands)
        DVE binary-counter tree sums the chunk's 16 P~^T pair tiles in
        whole [128,2,512] ops (bf16 2x mode, 16 DVE ops/chunk); a Pool
        partition_all_reduce turns the full-chunk sum into the denominator
        row D[lq] (no PE ones-matmul, no PSUM bank).
      Each chunk's tail is deferred into the next chunk's matmul stream:
      D -> per-partition scale via 4 tiny PE transposes + DVE reciprocal,
      MM4: out = A~ @ Wlast (f32r; PSUM shares one bank with the scale
      transposes), DVE: out*gamma*scale + x residual, per-tile DMAs out
      (overlapping the stt chain at the final tail).

Key engine-balance facts (CoreSim + HW iteration):
  - The ACT engine is the bottleneck: 16.7M softmax exps/core at ~0.83
    ns/elem/lane is a ~109 us floor, plus per-op overhead. Everything else
    is kept off the ACT queue in steady state (copies on DVE/Pool, DMAs on
    the SP/Pool queues).
  - The q/k path must stay f32 precision end to end: logits reach +-45, so
    bf16 q/k (0.4% relative) would inject +-0.18 logit noise ~= 2e-2 rel
    output error (measured in numpy simulation). bf16 is safe on the
    P~/v/A~ side (~1e-3); fp8 (DoubleRow) is NOT (top softmax weights get
    6% steps -> ~1.5e-2).
  - Per-chunk resident tiles (x/qT/kT/v split per 512-chunk) matter: tile
    dependencies are tracked per tile, so monolithic [128, L] tensors would
    serialize chunk-0 attention behind all eight phase-1 chunks.
  - Softmax skips max-subtraction: logits are O(+-45) so exp stays inside
    f32/bf16 range, and softmax is shift-invariant.
  - GPSIMD cannot touch PSUM (hardware verifier rule; CoreSim allows it),
    and f32r matmul operands must be produced by a rounding op, not a
    bitcast.

Measured: CoreSim 177.2 us (loop marginal 175.6); HW marginal time
~200-245 us/body across runs
(axon-tunnel timing noise is +-15-20%; best observed pair 201 us, medians
227-247 depending on machine load); the previous session's version
measured 250 us. rel err 1.2e-3 vs the fp32 reference (gate: 2e-2).
"""

import numpy as np

import concourse.bass as bass
import concourse.mybir as mybir
import concourse.tile as tile
from concourse import bacc
from concourse import bass_isa
from concourse.masks import make_identity
from concourse.bass_utils import run_bass_kernel_spmd

f32 = mybir.dt.float32
f32r = mybir.dt.float32r
bf16 = mybir.dt.bfloat16
AF = mybir.ActivationFunctionType
ALU = mybir.AluOpType

B, H, W, C, D = 8, 64, 64, 256, 128
L = H * W            # 4096
NT = L // 128        # 32 L-tiles of 128 rows
NCHUNK = L // 512    # 8 Lq chunks of 512
CK = C // 128        # 2 C-chunks


def _emit(nc, tc, ctx, nreps=1):
    x_d = nc.declare_dram_parameter("x", [L, C], f32, isOutput=False)
    wq_d = nc.declare_dram_parameter("Wq", [C, D], f32, isOutput=False)
    wk_d = nc.declare_dram_parameter("Wk", [C, D], f32, isOutput=False)
    wv_d = nc.declare_dram_parameter("Wv", [C, D], f32, isOutput=False)
    wl_d = nc.declare_dram_parameter("Wlast", [D, C], f32, isOutput=False)
    g_d = nc.declare_dram_parameter("gamma", [1], f32, isOutput=False)
    out_d = nc.declare_dram_parameter("out", [L, C], f32, isOutput=True)

    x_tiled = x_d[:].rearrange("(t p) c -> p t c", p=128)      # [128, NT, C]
    out_tiled = out_d[:].rearrange("(t p) c -> p t c", p=128)  # [128, NT, C]

    const = ctx.enter_context(tc.tile_pool(name="const", bufs=1))
    resident = ctx.enter_context(tc.tile_pool(name="resident", bufs=1))

    # --- constants -------------------------------------------------------
    identity = const.tile([128, 128], f32)
    make_identity(nc, identity[:])
    identity_h = const.tile([128, 128], bf16)
    nc.vector.tensor_copy(out=identity_h[:], in_=identity[:])
    id1 = const.tile([1, 1], f32)
    nc.vector.memset(id1[:], 1.0)
    gamma_sb = const.tile([128, 1], f32)
    nc.gpsimd.dma_start(out=gamma_sb[:], in_=g_d[:].to_broadcast((128, 1)))

    # weights: lhsT chunks [C128, D] for q/k/v, [D, C] for last
    w_r = {}
    for name, wd in (("q", wq_d), ("k", wk_d), ("v", wv_d)):
        wtmp = const.tile([128, CK, D], f32, name=f"wtmp_{name}")
        nc.gpsimd.dma_start(out=wtmp[:], in_=wd[:].rearrange("(cc p) d -> p cc d", p=128))
        wr = const.tile([128, CK, D], f32r, name=f"w_{name}")
        nc.vector.tensor_copy(out=wr[:], in_=wtmp[:])
        w_r[name] = wr
    wl_tmp = const.tile([128, C], f32)
    nc.gpsimd.dma_start(out=wl_tmp[:], in_=wl_d[:])
    wl_r = const.tile([128, C], f32r)
    nc.vector.tensor_copy(out=wl_r[:], in_=wl_tmp[:])

    if nreps == 1:
        _emit_body(nc, tc, const, resident, x_tiled, out_tiled,
                   identity, identity_h, id1, gamma_sb, w_r, wl_r)
    else:
        # dev-harness timing build: hardware loop re-running the identical
        # body (same inputs/outputs each iteration)
        with tc.For_i(0, nreps, 1):
            _emit_body(nc, tc, const, resident, x_tiled, out_tiled,
                       identity, identity_h, id1, gamma_sb, w_r, wl_r)


def _emit_body(nc, tc, const, resident, x_tiled, out_tiled,
               identity, identity_h, id1, gamma_sb, w_r, wl_r):
    NG = NT // 2  # 16 groups = pairs of Lk tiles per Lq chunk

    # --- resident tensors --------------------------------------------------
    # Everything L-indexed is split into per-chunk tiles: the tile framework
    # tracks dependencies per tile, so a single [128, L] tensor would make
    # every chunk-0 matmul wait on ALL eight phase-1 chunks. Per-chunk tiles
    # let chunk-c attention start as soon as its own producers finish.
    X_PIECES = ((0, 4), (4, 8), (8, 16), (16, 24), (24, 32))
    x_parts = {
        lo: resident.tile([128, hi - lo, C], f32, tag=f"x_sb{lo}",
                          name=f"x_sb{lo}")
        for lo, hi in X_PIECES
    }

    def x_tile(t):
        for lo, hi in X_PIECES:
            if lo <= t < hi:
                return x_parts[lo][:, t - lo, :]
        raise AssertionError(t)

    def load_x(lo, hi):
        nc.sync.dma_start(out=x_parts[lo][:], in_=x_tiled[:, lo:hi, :])

    load_x(0, 4)
    xt_parts = [resident.tile([128, CK, 512], f32r, tag=f"xt{p}",
                               name=f"xt{p}")
                for p in range(NCHUNK)]                      # 32 KB/part
    qT_parts = [resident.tile([128, 512], f32r, tag=f"qT{p}", name=f"qT{p}")
                for p in range(NCHUNK)]                      # 16 KB/part
    kT_parts = [resident.tile([128, 512], f32r, tag=f"kT{p}", name=f"kT{p}")
                for p in range(NCHUNK)]                      # 16 KB/part
    v_parts = [resident.tile([128, 4, D], bf16, tag=f"v{p}", name=f"v{p}")
               for p in range(NCHUNK)]                       # 8 KB/part

    def kT_tile(lk):
        return kT_parts[lk // 4][:, (lk % 4) * 128:(lk % 4 + 1) * 128]

    def v_tile(lk):
        return v_parts[lk // 4][:, lk % 4, :]

    with (
        tc.tile_pool(name="vt", bufs=3) as vtp,
        tc.tile_pool(name="pexp", bufs=16) as pexp,
        tc.tile_pool(name="psum2p", bufs=2) as psum2p,
        tc.tile_pool(name="dall", bufs=3) as dallp,
        tc.tile_pool(name="asb", bufs=3) as asb,
        tc.tile_pool(name="osb", bufs=3) as osb,
        tc.tile_pool(name="dsb", bufs=2) as dsb,
        tc.tile_pool(name="ps_s", bufs=2, space="PSUM") as ps_s,
        tc.tile_pool(name="ps_po", bufs=1, space="PSUM") as ps_po,
        tc.tile_pool(name="ps_acc", bufs=2, space="PSUM") as ps_acc,
    ):
        # --- phase 1 chunk: transposes + projections for L columns
        # [512p, 512p+512) -> qT/kT chunks, v tiles 4p..4p+3. Copies ride
        # the DVE/Pool so the ACT engine stays free for softmax exp; the
        # v^T -> v transpose rides the DMA xbar (2-byte dtype).
        def emit_xt_k(p):
            # pass A: x^T for this chunk (resident) + k projection. kT is
            # what gates chunk-0 attention, so it comes first for all
            # chunks; q/v follow in pass B using the resident x^T.
            xt_c = xt_parts[p]
            for cc in range(CK):
                ps = ps_s.tile([128, 512], f32, tag="s1")
                for i in range(4):
                    t = 4 * p + i
                    nc.tensor.transpose(
                        ps[:, i * 128:(i + 1) * 128],
                        x_tile(t)[:, cc * 128:(cc + 1) * 128],
                        identity[:],
                    )
                nc.scalar.activation(out=xt_c[:, cc, :], in_=ps[:],
                                     func=AF.Copy)
            names = (("k", kT_parts[p]),) if p else (
                ("k", kT_parts[p]), ("q", qT_parts[p]))
            for name, dstT in names:
                ps = ps_s.tile([128, 512], f32, tag="s1")
                for cc in range(CK):
                    nc.tensor.matmul(
                        ps[:], w_r[name][:, cc, :], xt_c[:, cc, :],
                        start=(cc == 0), stop=(cc == CK - 1),
                    )
                nc.vector.tensor_copy(out=dstT[:], in_=ps[:])

        def emit_qv(p):
            xt_c = xt_parts[p]
            if p:
                ps = ps_s.tile([128, 512], f32, tag="s1")
                for cc in range(CK):
                    nc.tensor.matmul(
                        ps[:], w_r["q"][:, cc, :], xt_c[:, cc, :],
                        start=(cc == 0), stop=(cc == CK - 1),
                    )
                nc.vector.tensor_copy(out=qT_parts[p][:], in_=ps[:])
            ps = ps_s.tile([128, 512], f32, tag="s1")
            for cc in range(CK):
                nc.tensor.matmul(
                    ps[:], w_r["v"][:, cc, :], xt_c[:, cc, :],
                    start=(cc == 0), stop=(cc == CK - 1),
                )
            vt_c = vtp.tile([128, 512], bf16)
            nc.scalar.activation(out=vt_c[:], in_=ps[:], func=AF.Copy)
            ps2 = ps_po.tile([128, 512], bf16, tag="po", name="vtr")
            for i in range(4):
                nc.tensor.transpose(
                    ps2[:, i * 128:(i + 1) * 128],
                    vt_c[:, i * 128:(i + 1) * 128], identity_h[:],
                )
            nc.vector.tensor_copy(out=v_parts[p][:], in_=ps2[:])

        def emit_tail(c, acc, dall):
            # A~^T to SBUF for MM4. DVE while ACT is the steady-state wall;
            # the last chunk's copy rides the then-idle ACT instead, which
            # shortens the end-of-body drain chain.
            a_sb = asb.tile([128, 512], f32r, tag="a_sb", name="a_sb")
            if c == NCHUNK - 1:
                nc.scalar.activation(out=a_sb[:], in_=acc[:], func=AF.Copy)
            else:
                nc.vector.tensor_copy(out=a_sb[:], in_=acc[:])
            # dall holds the denominator row D[lq] on every partition;
            # transpose 128-blocks of partition 0 to per-partition columns
            ps_sc = ps_po.tile([128, 4], f32, tag="po", name="ps_sc")
            for m in range(4):
                nc.tensor.transpose(
                    ps_sc[:, m:m + 1], dall[0:1, m * 128:(m + 1) * 128], id1[:]
                )
            sc_raw = dsb.tile([128, 4], f32, tag="scraw", name="scraw")
            nc.vector.tensor_copy(out=sc_raw[:], in_=ps_sc[:])
            sc = dsb.tile([128, 4], f32, tag="sc", name="sc")
            nc.vector.reciprocal(out=sc[:], in_=sc_raw[:])
            nc.vector.tensor_scalar_mul(sc[:], sc[:], gamma_sb[:])

            o_sb = osb.tile([128, 4, C], f32, tag="o_sb", name="o_sb")
            for m in range(4):
                t = 4 * c + m
                # MM4 output shares the S-pair slot rotation
                po = ps_po.tile([128, C], f32, tag="po", name="po")
                nc.tensor.matmul(
                    po[:], a_sb[:, m * 128:(m + 1) * 128], wl_r[:],
                    start=True, stop=True,
                )
                nc.vector.scalar_tensor_tensor(
                    out=o_sb[:, m, :], in0=po[:], scalar=sc[:, m:m + 1],
                    in1=x_tile(t), op0=ALU.mult, op1=ALU.add,
                )
                nc.sync.dma_start(
                    out=out_tiled[:, t:t + 1, :], in_=o_sb[:, m:m + 1, :]
                )

        # --- phase 2 machinery: one global software pipe across all chunks
        # (no per-chunk drain: the PE stream flows MM1(c+1) right after
        # MM2(c), and each chunk's tail is deferred into the next chunk's
        # matmul stream).
        accs, dalls = {}, {}
        tree_state = {}
        pipe = []
        pending_tail = [None]

        def emit_pair(c, g):
            if g == 0:
                accs[c] = ps_acc.tile([128, 512], f32, tag="acc", name=f"acc{c}")
            # S^T pair: two MM1s into one 2-bank PSUM tile, ONE exp op
            sp = ps_s.tile([128, 2, 512], f32, tag="s1", name="s1")
            for j in range(2):
                lk = 2 * g + j
                nc.tensor.matmul(
                    sp[:, j, :], kT_tile(lk),
                    qT_parts[c][:], start=True, stop=True,
                )
            p_pair = pexp.tile([128, 2, 512], bf16, tag="p1", name="p1")
            nc.scalar.activation(out=p_pair[:], in_=sp[:], func=AF.Exp)
            # denominator reduction: binary-counter tree over WHOLE pair
            # tiles (the tree only feeds ff, so any association order is
            # fine; [128,2,512] ops halve the DVE instruction count), then
            # a Pool partition_all_reduce replaces the ones-matmul (and its
            # PSUM bank) entirely.
            t = p_pair
            lvl = 0
            while tree_state.get(lvl) is not None:
                prev = tree_state.pop(lvl)
                nxt = psum2p.tile([128, 2, 512], bf16, tag=f"t{lvl}",
                                  name=f"t{lvl}")
                nc.vector.tensor_tensor(
                    out=nxt[:], in0=prev[:], in1=t[:], op=ALU.add,
                )
                t = nxt
                lvl += 1
            if g == NG - 1:
                # t holds two 16-tile sums; fold and all-reduce partitions
                ff = psum2p.tile([128, 512], f32, tag="ff", name="ff")
                nc.vector.tensor_tensor(
                    out=ff[:], in0=t[:, 0, :], in1=t[:, 1, :], op=ALU.add,
                )
                dall = dallp.tile([128, 512], f32, tag="dall",
                                  name=f"dall{c}")
                nc.gpsimd.partition_all_reduce(
                    dall[:], ff[:], channels=128,
                    reduce_op=bass_isa.ReduceOp.add,
                )
                dalls[c] = dall
                tree_state.clear()
            else:
                tree_state[lvl] = t
            return (c, g, p_pair)

        def pop_one():
            c, g, p_pair = pipe.pop(0)
            acc = accs[c]
            for j in range(2):
                lk = 2 * g + j
                nc.tensor.matmul(
                    acc[:], v_tile(lk), p_pair[:, j, :],
                    start=(lk == 0), stop=(lk == NT - 1),
                    skip_group_check=True,
                )
            if g == NG - 1:
                pending_tail[0] = (c, acc, dalls[c])
            elif g == 1 and pending_tail[0] is not None:
                # previous chunk's tail: far enough into this chunk that the
                # Pool/DVE have caught up, early enough to stay off the
                # critical path
                emit_tail(*pending_tail[0])
                pending_tail[0] = None

        def push(item, limit=3):
            pipe.append(item)
            while len(pipe) > limit:
                pop_one()

        # --- emission: phase-1 chunks interleaved with chunk-0 attention.
        # Pair (0, g) needs kT/v tiles 2g..2g+1 == phase-1 chunk g//2, so
        # after phase-1 chunk p the pairs 2(p-1)-2.. of chunk 0 are safe.
        x_pieces = [(4, 8), (8, 16), (16, 24), (24, 32)]
        emit_xt_k(0)
        for p in range(1, NCHUNK):
            if x_pieces:
                load_x(*x_pieces.pop(0))
            emit_xt_k(p)
        for p in range(NCHUNK):
            emit_qv(p)
        for g in range(NG):
            push(emit_pair(0, g), limit=NG)
        n_push = 0
        for c in range(1, NCHUNK):
            for g in range(NG):
                if c == NCHUNK - 1 and g >= NG - 3:
                    lim = 1
                else:
                    lim = max(3, NG - n_push // 6)
                push(emit_pair(c, g), limit=lim)
                n_push += 1
        while pipe:
            pop_one()
        emit_tail(*pending_tail[0])


_NC_CACHE = {}


def _build(nreps=1):
    """Build the Bass module; nreps>1 repeats the whole body (for marginal-
    time measurement in the dev harness — grading path uses nreps=1)."""
    if nreps not in _NC_CACHE:
        from contextlib import ExitStack

        nc = bacc.Bacc("TRN2", target_bir_lowering=False)
        with tile.TileContext(nc) as tc:
            with ExitStack() as ctx:
                _emit(nc, tc, ctx, nreps=nreps)
        nc.compile()
        _NC_CACHE[nreps] = nc
    return _NC_CACHE[nreps]


def kernel(x, Wq, Wk, Wv, Wlast, gamma):
    assert x.shape == (B, H, W, C), x.shape
    nc = _build()
    xf = np.ascontiguousarray(x, dtype=np.float32).reshape(B, L, C)
    in_maps = [
        {
            "x": xf[b],
            "Wq": np.ascontiguousarray(Wq, dtype=np.float32),
            "Wk": np.ascontiguousarray(Wk, dtype=np.float32),
            "Wv": np.ascontiguousarray(Wv, dtype=np.float32),
            "Wlast": np.ascontiguousarray(Wlast, dtype=np.float32),
            "gamma": np.ascontiguousarray(gamma, dtype=np.float32),
        }
        for b in range(B)
    ]
    res = run_bass_kernel_spmd(nc, in_maps, core_ids=list(range(B)))
    out = np.stack([res.results[b]["out"] for b in range(B)], axis=0)
    return out.reshape(B, H, W, C)


# revision 34
# speedup vs baseline: 1.2744x; 1.2744x over previous
"""Trainium2 Bass kernel for nn_AttentionLayer (B=8, H=W=64, C=256, D=128).

Strategy: data-parallel over batch B=8 across the 8 NeuronCores (attention is
independent per batch element). Per core, for its batch element's x [L=4096,
C=256]:

  phase 1, two passes: pass A PE-transposes x tiles (f32) -> resident xT
      and projects k^T [D, 512] (f32r) for ALL chunks first (kT is what
      gates chunk-0 attention); pass B projects q^T and v^T from the
      resident xT (v^T -> bf16 -> PE-transposed v tiles). PSUM->SBUF xT/vT
      copies ride the ACT engine (idle during the prologue); q/k copies
      ride the DVE.
  phase 2 (per 512-wide Lq chunk, per PAIR of 128-row Lk tiles, one global
      software pipeline across all chunks):
        MM1: S^T pair   = k_tile @ q_chunk^T x2   (PE, f32r, into a 2-bank
                          PSUM pair tile)
        exp: P~^T pair  = exp(S^T pair)           (ONE ACT op per pair ->
                          bf16; halves the ACT instruction count, which is
                          the bottleneck engine)
        MM2: A~^T      += v_tile^T @ P~^T tile    (PE, bf16 operands)
        DVE binary-counter tree sums the chunk's 16 P~^T pair tiles in
        whole [128,2,512] ops (bf16 2x mode, 16 DVE ops/chunk); a Pool
        partition_all_reduce turns the full-chunk sum into the denominator
        row D[lq] (no PE ones-matmul, no PSUM bank).
      Each chunk's tail is deferred into the next chunk's matmul stream:
      D -> per-partition scale via 4 tiny PE transposes + DVE reciprocal,
      MM4: out = A~ @ Wlast (f32r; PSUM shares one bank with the scale
      transposes), DVE: out*gamma*scale + x residual, per-tile DMAs out
      (overlapping the stt chain at the final tail).

Key engine-balance facts (CoreSim + HW iteration):
  - The ACT engine is the bottleneck: 16.7M softmax exps/core at ~0.83
    ns/elem/lane is a ~109 us floor, plus per-op overhead. Everything else
    is kept off the ACT queue in steady state (copies on DVE/Pool, DMAs on
    the SP/Pool queues).
  - The q/k path must stay f32 precision end to end: logits reach +-45, so
    bf16 q/k (0.4% relative) would inject +-0.18 logit noise ~= 2e-2 rel
    output error (measured in numpy simulation). bf16 is safe on the
    P~/v/A~ side (~1e-3); fp8 (DoubleRow) is NOT (top softmax weights get
    6% steps -> ~1.5e-2).
  - Per-chunk resident tiles (x/qT/kT/v split per 512-chunk) matter: tile
    dependencies are tracked per tile, so monolithic [128, L] tensors would
    serialize chunk-0 attention behind all eight phase-1 chunks.
  - Softmax skips max-subtraction: logits are O(+-45) so exp stays inside
    f32/bf16 range, and softmax is shift-invariant.
  - GPSIMD cannot touch PSUM (hardware verifier rule; CoreSim allows it),
    and f32r matmul operands must be produced by a rounding op, not a
    bitcast.

Measured: CoreSim 177.2 us (loop marginal 175.6); HW marginal time
~200-245 us/body across runs
(axon-tunnel timing noise is +-15-20%; best observed pair 201 us, medians
227-247 depending on machine load); the previous session's version
measured 250 us. rel err 1.2e-3 vs the fp32 reference (gate: 2e-2).
"""

import numpy as np

import concourse.bass as bass
import concourse.mybir as mybir
import concourse.tile as tile
from concourse import bacc
from concourse import bass_isa
from concourse.masks import make_identity
from concourse.bass_utils import run_bass_kernel_spmd

f32 = mybir.dt.float32
f32r = mybir.dt.float32r
bf16 = mybir.dt.bfloat16
AF = mybir.ActivationFunctionType
ALU = mybir.AluOpType

B, H, W, C, D = 8, 64, 64, 256, 128
L = H * W            # 4096
NT = L // 128        # 32 L-tiles of 128 rows
NCHUNK = L // 512    # 8 Lq chunks of 512
CK = C // 128        # 2 C-chunks


def _emit(nc, tc, ctx, nreps=1):
    x_d = nc.declare_dram_parameter("x", [L, C], f32, isOutput=False)
    wq_d = nc.declare_dram_parameter("Wq", [C, D], f32, isOutput=False)
    wk_d = nc.declare_dram_parameter("Wk", [C, D], f32, isOutput=False)
    wv_d = nc.declare_dram_parameter("Wv", [C, D], f32, isOutput=False)
    wl_d = nc.declare_dram_parameter("Wlast", [D, C], f32, isOutput=False)
    g_d = nc.declare_dram_parameter("gamma", [1], f32, isOutput=False)
    out_d = nc.declare_dram_parameter("out", [L, C], f32, isOutput=True)

    x_tiled = x_d[:].rearrange("(t p) c -> p t c", p=128)      # [128, NT, C]
    out_tiled = out_d[:].rearrange("(t p) c -> p t c", p=128)  # [128, NT, C]

    const = ctx.enter_context(tc.tile_pool(name="const", bufs=1))
    resident = ctx.enter_context(tc.tile_pool(name="resident", bufs=1))

    # --- constants -------------------------------------------------------
    identity = const.tile([128, 128], f32)
    make_identity(nc, identity[:])
    identity_h = const.tile([128, 128], bf16)
    nc.vector.tensor_copy(out=identity_h[:], in_=identity[:])
    id1 = const.tile([1, 1], f32)
    nc.vector.memset(id1[:], 1.0)
    gamma_sb = const.tile([128, 1], f32)
    nc.gpsimd.dma_start(out=gamma_sb[:], in_=g_d[:].to_broadcast((128, 1)))

    # weights: lhsT chunks [C128, D] for q/k/v, [D, C] for last
    w_r = {}
    for name, wd in (("q", wq_d), ("k", wk_d), ("v", wv_d)):
        wtmp = const.tile([128, CK, D], f32, name=f"wtmp_{name}")
        nc.gpsimd.dma_start(out=wtmp[:], in_=wd[:].rearrange("(cc p) d -> p cc d", p=128))
        wr = const.tile([128, CK, D], f32r, name=f"w_{name}")
        nc.vector.tensor_copy(out=wr[:], in_=wtmp[:])
        w_r[name] = wr
    wl_tmp = const.tile([128, C], f32)
    nc.gpsimd.dma_start(out=wl_tmp[:], in_=wl_d[:])
    wl_r = const.tile([128, C], f32r)
    nc.vector.tensor_copy(out=wl_r[:], in_=wl_tmp[:])

    if nreps == 1:
        _emit_body(nc, tc, const, resident, x_tiled, out_tiled,
                   identity, identity_h, id1, gamma_sb, w_r, wl_r)
    else:
        # dev-harness timing build: hardware loop re-running the identical
        # body (same inputs/outputs each iteration)
        with tc.For_i(0, nreps, 1):
            _emit_body(nc, tc, const, resident, x_tiled, out_tiled,
                       identity, identity_h, id1, gamma_sb, w_r, wl_r)


def _emit_body(nc, tc, const, resident, x_tiled, out_tiled,
               identity, identity_h, id1, gamma_sb, w_r, wl_r):
    NG = NT // 2  # 16 groups = pairs of Lk tiles per Lq chunk

    # --- resident tensors --------------------------------------------------
    # Everything L-indexed is split into per-chunk tiles: the tile framework
    # tracks dependencies per tile, so a single [128, L] tensor would make
    # every chunk-0 matmul wait on ALL eight phase-1 chunks. Per-chunk tiles
    # let chunk-c attention start as soon as its own producers finish.
    X_PIECES = ((0, 4), (4, 8), (8, 16), (16, 24), (24, 32))
    x_parts = {
        lo: resident.tile([128, hi - lo, C], f32, tag=f"x_sb{lo}",
                          name=f"x_sb{lo}")
        for lo, hi in X_PIECES
    }

    def x_tile(t):
        for lo, hi in X_PIECES:
            if lo <= t < hi:
                return x_parts[lo][:, t - lo, :]
        raise AssertionError(t)

    def load_x(lo, hi):
        nc.sync.dma_start(out=x_parts[lo][:], in_=x_tiled[:, lo:hi, :])

    load_x(0, 4)
    xt_parts = [resident.tile([128, CK, 512], f32r, tag=f"xt{p}",
                               name=f"xt{p}")
                for p in range(NCHUNK)]                      # 32 KB/part
    qT_parts = [resident.tile([128, 512], f32r, tag=f"qT{p}", name=f"qT{p}")
                for p in range(NCHUNK)]                      # 16 KB/part
    kT_parts = [resident.tile([128, 512], f32r, tag=f"kT{p}", name=f"kT{p}")
                for p in range(NCHUNK)]                      # 16 KB/part
    v_parts = [resident.tile([128, 4, D], bf16, tag=f"v{p}", name=f"v{p}")
               for p in range(NCHUNK)]                       # 8 KB/part

    def kT_tile(lk):
        return kT_parts[lk // 4][:, (lk % 4) * 128:(lk % 4 + 1) * 128]

    def v_tile(lk):
        return v_parts[lk // 4][:, lk % 4, :]

    with (
        tc.tile_pool(name="vt", bufs=3) as vtp,
        tc.tile_pool(name="pexp", bufs=16) as pexp,
        tc.tile_pool(name="psum2p", bufs=2) as psum2p,
        tc.tile_pool(name="dall", bufs=3) as dallp,
        tc.tile_pool(name="asb", bufs=3) as asb,
        tc.tile_pool(name="osb", bufs=3) as osb,
        tc.tile_pool(name="dsb", bufs=2) as dsb,
        tc.tile_pool(name="ps_s", bufs=2, space="PSUM") as ps_s,
        tc.tile_pool(name="ps_p1", bufs=2, space="PSUM") as ps_p1,
        tc.tile_pool(name="ps_po", bufs=1, space="PSUM") as ps_po,
        tc.tile_pool(name="ps_acc", bufs=1, space="PSUM") as ps_acc,
    ):
        # --- phase 1 chunk: transposes + projections for L columns
        # [512p, 512p+512) -> qT/kT chunks, v tiles 4p..4p+3. Copies ride
        # the DVE/Pool so the ACT engine stays free for softmax exp; the
        # v^T -> v transpose rides the DMA xbar (2-byte dtype).
        def emit_xt_k(p):
            # pass A: x^T for this chunk (resident) + k projection. kT is
            # what gates chunk-0 attention, so it comes first for all
            # chunks; q/v follow in pass B using the resident x^T.
            xt_c = xt_parts[p]
            for cc in range(CK):
                ps = ps_p1.tile([128, 512], f32, tag="p1ps")
                for i in range(4):
                    t = 4 * p + i
                    nc.tensor.transpose(
                        ps[:, i * 128:(i + 1) * 128],
                        x_tile(t)[:, cc * 128:(cc + 1) * 128],
                        identity[:],
                    )
                nc.scalar.activation(out=xt_c[:, cc, :], in_=ps[:],
                                     func=AF.Copy)
            names = (("k", kT_parts[p]),) if p else (
                ("k", kT_parts[p]), ("q", qT_parts[p]))
            for name, dstT in names:
                ps = ps_p1.tile([128, 512], f32, tag="p1ps")
                for cc in range(CK):
                    nc.tensor.matmul(
                        ps[:], w_r[name][:, cc, :], xt_c[:, cc, :],
                        start=(cc == 0), stop=(cc == CK - 1),
                    )
                nc.vector.tensor_copy(out=dstT[:], in_=ps[:])

        def emit_qv(p):
            xt_c = xt_parts[p]
            if p:
                ps = ps_p1.tile([128, 512], f32, tag="p1ps")
                for cc in range(CK):
                    nc.tensor.matmul(
                        ps[:], w_r["q"][:, cc, :], xt_c[:, cc, :],
                        start=(cc == 0), stop=(cc == CK - 1),
                    )
                nc.vector.tensor_copy(out=qT_parts[p][:], in_=ps[:])
            ps = ps_p1.tile([128, 512], f32, tag="p1ps")
            for cc in range(CK):
                nc.tensor.matmul(
                    ps[:], w_r["v"][:, cc, :], xt_c[:, cc, :],
                    start=(cc == 0), stop=(cc == CK - 1),
                )
            vt_c = vtp.tile([128, 512], bf16)
            nc.scalar.activation(out=vt_c[:], in_=ps[:], func=AF.Copy)
            ps2 = ps_po.tile([128, 512], bf16, tag="po", name="vtr")
            for i in range(4):
                nc.tensor.transpose(
                    ps2[:, i * 128:(i + 1) * 128],
                    vt_c[:, i * 128:(i + 1) * 128], identity_h[:],
                )
            nc.vector.tensor_copy(out=v_parts[p][:], in_=ps2[:])

        def emit_tail(c, a_sb, dall):
            # dall holds the denominator row D[lq] on every partition;
            # transpose 128-blocks of partition 0 to per-partition columns
            ps_sc = ps_po.tile([128, 4], f32, tag="po", name="ps_sc")
            for m in range(4):
                nc.tensor.transpose(
                    ps_sc[:, m:m + 1], dall[0:1, m * 128:(m + 1) * 128], id1[:]
                )
            sc_raw = dsb.tile([128, 4], f32, tag="scraw", name="scraw")
            nc.vector.tensor_copy(out=sc_raw[:], in_=ps_sc[:])
            sc = dsb.tile([128, 4], f32, tag="sc", name="sc")
            nc.vector.reciprocal(out=sc[:], in_=sc_raw[:])
            nc.vector.tensor_scalar_mul(sc[:], sc[:], gamma_sb[:])

            o_sb = osb.tile([128, 4, C], f32, tag="o_sb", name="o_sb")
            for m in range(4):
                t = 4 * c + m
                # MM4 output shares the S-pair slot rotation
                po = ps_po.tile([128, C], f32, tag="po", name="po")
                nc.tensor.matmul(
                    po[:], a_sb[:, m * 128:(m + 1) * 128], wl_r[:],
                    start=True, stop=True,
                )
                nc.vector.scalar_tensor_tensor(
                    out=o_sb[:, m, :], in0=po[:], scalar=sc[:, m:m + 1],
                    in1=x_tile(t), op0=ALU.mult, op1=ALU.add,
                )
                nc.sync.dma_start(
                    out=out_tiled[:, t:t + 1, :], in_=o_sb[:, m:m + 1, :]
                )

        # --- phase 2 machinery: one global software pipe across all chunks
        # (no per-chunk drain: the PE stream flows MM1(c+1) right after
        # MM2(c), and each chunk's tail is deferred into the next chunk's
        # matmul stream).
        accs, dalls = {}, {}
        tree_state = {}
        pipe = []
        pending_tail = [None]

        def emit_pair(c, g):
            if g == 0:
                accs[c] = ps_acc.tile([128, 512], f32, tag="acc", name=f"acc{c}")
            # S^T pair: two MM1s into one 2-bank PSUM tile, ONE exp op
            sp = ps_s.tile([128, 2, 512], f32, tag="s1", name="s1")
            for j in range(2):
                lk = 2 * g + j
                nc.tensor.matmul(
                    sp[:, j, :], kT_tile(lk),
                    qT_parts[c][:], start=True, stop=True,
                )
            p_pair = pexp.tile([128, 2, 512], bf16, tag="p1", name="p1")
            nc.scalar.activation(out=p_pair[:], in_=sp[:], func=AF.Exp)
            # denominator reduction: binary-counter tree over WHOLE pair
            # tiles (the tree only feeds ff, so any association order is
            # fine; [128,2,512] ops halve the DVE instruction count), then
            # a Pool partition_all_reduce replaces the ones-matmul (and its
            # PSUM bank) entirely.
            t = p_pair
            lvl = 0
            while tree_state.get(lvl) is not None:
                prev = tree_state.pop(lvl)
                nxt = psum2p.tile([128, 2, 512], bf16, tag=f"t{lvl}",
                                  name=f"t{lvl}")
                nc.vector.tensor_tensor(
                    out=nxt[:], in0=prev[:], in1=t[:], op=ALU.add,
                )
                t = nxt
                lvl += 1
            if g == NG - 1:
                # t holds two 16-tile sums; fold and all-reduce partitions
                ff = psum2p.tile([128, 512], f32, tag="ff", name="ff")
                nc.vector.tensor_tensor(
                    out=ff[:], in0=t[:, 0, :], in1=t[:, 1, :], op=ALU.add,
                )
                dall = dallp.tile([128, 512], f32, tag="dall",
                                  name=f"dall{c}")
                nc.gpsimd.partition_all_reduce(
                    dall[:], ff[:], channels=128,
                    reduce_op=bass_isa.ReduceOp.add,
                )
                dalls[c] = dall
                tree_state.clear()
            else:
                tree_state[lvl] = t
            return (c, g, p_pair)

        def pop_one():
            c, g, p_pair = pipe.pop(0)
            acc = accs[c]
            for j in range(2):
                lk = 2 * g + j
                nc.tensor.matmul(
                    acc[:], v_tile(lk), p_pair[:, j, :],
                    start=(lk == 0), stop=(lk == NT - 1),
                    skip_group_check=True,
                )
            if g == NG - 1:
                # A~^T to SBUF immediately so the single acc bank frees
                # before the next chunk's MM2s reach it. DVE while ACT is
                # the steady-state wall; the last chunk's copy rides the
                # then-idle ACT, shortening the end-of-body drain chain.
                a_sb = asb.tile([128, 512], f32r, tag="a_sb", name="a_sb")
                if c == NCHUNK - 1:
                    nc.scalar.activation(out=a_sb[:], in_=acc[:],
                                         func=AF.Copy)
                else:
                    nc.vector.tensor_copy(out=a_sb[:], in_=acc[:])
                pending_tail[0] = (c, a_sb, dalls[c])
            elif g == 1 and pending_tail[0] is not None:
                # previous chunk's tail: far enough into this chunk that the
                # Pool/DVE have caught up, early enough to stay off the
                # critical path
                emit_tail(*pending_tail[0])
                pending_tail[0] = None

        def push(item, limit=3):
            pipe.append(item)
            while len(pipe) > limit:
                pop_one()

        # --- emission: phase-1 chunks interleaved with chunk-0 attention.
        # Pair (0, g) needs kT/v tiles 2g..2g+1 == phase-1 chunk g//2, so
        # after phase-1 chunk p the pairs 2(p-1)-2.. of chunk 0 are safe.
        x_pieces = [(4, 8), (8, 16), (16, 24), (24, 32)]
        emit_xt_k(0)
        for p in range(1, NCHUNK):
            if x_pieces:
                load_x(*x_pieces.pop(0))
            emit_xt_k(p)
        for p in range(NCHUNK):
            emit_qv(p)
        for g in range(NG):
            push(emit_pair(0, g), limit=NG)
        n_push = 0
        for c in range(1, NCHUNK):
            for g in range(NG):
                if c == NCHUNK - 1 and g >= NG - 3:
                    lim = 1
                else:
                    lim = max(3, NG - n_push // 6)
                push(emit_pair(c, g), limit=lim)
                n_push += 1
        while pipe:
            pop_one()
        emit_tail(*pending_tail[0])


_NC_CACHE = {}


def _build(nreps=1):
    """Build the Bass module; nreps>1 repeats the whole body (for marginal-
    time measurement in the dev harness — grading path uses nreps=1)."""
    if nreps not in _NC_CACHE:
        from contextlib import ExitStack

        nc = bacc.Bacc("TRN2", target_bir_lowering=False)
        with tile.TileContext(nc) as tc:
            with ExitStack() as ctx:
                _emit(nc, tc, ctx, nreps=nreps)
        nc.compile()
        _NC_CACHE[nreps] = nc
    return _NC_CACHE[nreps]


def kernel(x, Wq, Wk, Wv, Wlast, gamma):
    assert x.shape == (B, H, W, C), x.shape
    nc = _build()
    xf = np.ascontiguousarray(x, dtype=np.float32).reshape(B, L, C)
    in_maps = [
        {
            "x": xf[b],
            "Wq": np.ascontiguousarray(Wq, dtype=np.float32),
            "Wk": np.ascontiguousarray(Wk, dtype=np.float32),
            "Wv": np.ascontiguousarray(Wv, dtype=np.float32),
            "Wlast": np.ascontiguousarray(Wlast, dtype=np.float32),
            "gamma": np.ascontiguousarray(gamma, dtype=np.float32),
        }
        for b in range(B)
    ]
    res = run_bass_kernel_spmd(nc, in_maps, core_ids=list(range(B)))
    out = np.stack([res.results[b]["out"] for b in range(B)], axis=0)
    return out.reshape(B, H, W, C)
